# revision 15
# baseline (speedup 1.0000x reference)
"""Trainium2 Bass kernel for the DCN cross layer.

Computes out = x0 * (x_cross @ w)[:, None] + b + x_cross for
x0, x_cross: [16384, 4096] f32, w, b: [4096] f32.

Sharding: pure data parallel — batch split across 8 NeuronCores,
w and b replicated. Each core processes a [2048, 4096] shard.
"""

import sys

import numpy as np

sys.path.insert(0, "/opt/trn_rl_repo")

N_CORES = 8
BATCH = 16384
D = 4096
ROWS_PER_CORE = BATCH // N_CORES  # 2048
P = 128
RPP = 1  # rows per partition per tile -> DMA transfer size = RPP * 2 MB
N_TILES = ROWS_PER_CORE // (P * RPP)
BUFS = 3

_NC = None


def _build():
    """Build + schedule the single-core SPMD program (same on all cores)."""
    from contextlib import ExitStack

    import concourse.tile as tile
    from concourse import bacc, mybir

    f32 = mybir.dt.float32
    mult = mybir.AluOpType.mult
    add = mybir.AluOpType.add

    nc = bacc.Bacc(
        "TRN2", target_bir_lowering=False, debug=False, num_devices=N_CORES
    )
    x0_d = nc.dram_tensor("x0", [ROWS_PER_CORE, D], f32, kind="ExternalInput").ap()
    xc_d = nc.dram_tensor(
        "x_cross", [ROWS_PER_CORE, D], f32, kind="ExternalInput"
    ).ap()
    w_d = nc.dram_tensor("w", [D], f32, kind="ExternalInput").ap()
    b_d = nc.dram_tensor("b", [D], f32, kind="ExternalInput").ap()
    out_d = nc.dram_tensor("out", [ROWS_PER_CORE, D], f32, kind="ExternalOutput").ap()

    rows_per_tile = P * RPP
    with tile.TileContext(nc) as tc, ExitStack() as ctx:
        consts = ctx.enter_context(tc.tile_pool(name="consts", bufs=1))
        xc_pool = ctx.enter_context(tc.tile_pool(name="xc", bufs=BUFS))
        x0_pool = ctx.enter_context(tc.tile_pool(name="x0", bufs=BUFS))
        # tmp is produced and consumed only by DVE (in-order) -> 1 buf is free
        tmp_pool = ctx.enter_context(tc.tile_pool(name="tmp", bufs=1))
        out_pool = ctx.enter_context(tc.tile_pool(name="outp", bufs=2))
        s_pool = ctx.enter_context(tc.tile_pool(name="s", bufs=4))

        # w and b replicated across all 128 partitions (one-time). The
        # stride-0 DMA broadcast re-reads the same 16 KB per partition but
        # overlaps with the load stream and beat gpsimd.partition_broadcast
        # by ~8 us end-to-end.
        w_t = consts.tile([P, D], f32)
        b_t = consts.tile([P, D], f32)
        # issue on the ACT ring (stores come much later there) so the SP
        # ring starts streaming x0/x_cross immediately
        nc.scalar.dma_start(out=w_t[:], in_=w_d.partition_broadcast(P))
        nc.scalar.dma_start(out=b_t[:], in_=b_d.partition_broadcast(P))

        for i in range(N_TILES):
            r0 = i * rows_per_tile
            # [rows_per_tile, D] DRAM block == [P, RPP*D] SBUF tile
            # (partition p holds rows r0 + RPP*p .. r0 + RPP*p + RPP-1)
            xc_t = xc_pool.tile([P, RPP * D], f32)
            nc.sync.dma_start(
                out=xc_t[:],
                in_=xc_d[r0 : r0 + rows_per_tile, :].rearrange(
                    "(p r) d -> p (r d)", p=P
                ),
            )
            x0_t = x0_pool.tile([P, RPP * D], f32)
            nc.sync.dma_start(
                out=x0_t[:],
                in_=x0_d[r0 : r0 + rows_per_tile, :].rearrange(
                    "(p r) d -> p (r d)", p=P
                ),
            )

            tmp_t = tmp_pool.tile([P, D], f32)
            o_t = out_pool.tile([P, RPP * D], f32)
            s_t = s_pool.tile([P, RPP], f32)
            for j in range(RPP):
                ds = slice(j * D, (j + 1) * D)
                # tmp = xc * w (junk), s = rowsum(xc * w)
                # (tensor_tensor_reduce's native opcode crashes this runtime;
                # scalar_tensor_tensor's accum_out path does the same thing)
                nc.vector.scalar_tensor_tensor(
                    out=tmp_t[:],
                    in0=xc_t[:, ds],
                    scalar=1.0,
                    in1=w_t[:],
                    op0=mult,
                    op1=mult,
                    accum_out=s_t[:, j : j + 1],
                )
                # tmp = x0 * s + xc
                nc.vector.scalar_tensor_tensor(
                    out=tmp_t[:],
                    in0=x0_t[:, ds],
                    scalar=s_t[:, j : j + 1],
                    in1=xc_t[:, ds],
                    op0=mult,
                    op1=add,
                )
                # out = tmp + b into a dedicated output slot, so the xc/x0
                # slots free as soon as compute has read them (not after the
                # store drains)
                nc.vector.tensor_add(o_t[:, ds], tmp_t[:], b_t[:])
            # store from the ACT HWDGE ring so loads (SP ring) and stores
            # use separate descriptor generators
            nc.scalar.dma_start(
                out=out_d[r0 : r0 + rows_per_tile, :].rearrange(
                    "(p r) d -> p (r d)", p=P
                ),
                in_=o_t[:],
            )

    nc.compile()
    return nc


def _get_nc():
    global _NC
    if _NC is None:
        _NC = _build()
    return _NC


def _run(inputs, trace=False, **spmd_kwargs):
    """Shard, run on 8 cores, gather. Returns (full_output, BassKernelResults)."""
    from concourse.bass_utils import run_bass_kernel_spmd

    nc = _get_nc()

    x0 = np.ascontiguousarray(np.asarray(inputs["x0"], dtype=np.float32))
    xc = np.ascontiguousarray(np.asarray(inputs["x_cross"], dtype=np.float32))
    w = np.ascontiguousarray(np.asarray(inputs["w"], dtype=np.float32))
    b = np.ascontiguousarray(np.asarray(inputs["b"], dtype=np.float32))

    in_maps = [
        {
            "x0": x0[i * ROWS_PER_CORE : (i + 1) * ROWS_PER_CORE],
            "x_cross": xc[i * ROWS_PER_CORE : (i + 1) * ROWS_PER_CORE],
            "w": w,
            "b": b,
        }
        for i in range(N_CORES)
    ]

    res = run_bass_kernel_spmd(
        nc, in_maps, core_ids=list(range(N_CORES)), trace=trace, **spmd_kwargs
    )
    out = np.concatenate([res.results[i]["out"] for i in range(N_CORES)], axis=0)
    return out, res


def kernel(**inputs: np.ndarray) -> np.ndarray:
    out, _ = _run(inputs)
    return out
